# revision 10
# baseline (speedup 1.0000x reference)
"""GPT forward (embed + 8-head causal attn w/ query-axis softmax + lm_head + CE loss)
on 8 Trainium2 NeuronCores.

Sharding: attention is sharded by (batch, head) pairs (2 per core); the head
outputs are all-gathered on device; the lm_head / logits / log-sum-exp are
sharded over the vocab axis (6400 padded rows per core). Host only
slices/concatenates shards and combines the per-shard sum-exp scalars.
"""

import sys

sys.path.insert(0, "/opt/trn_rl_repo")

import numpy as np
import ml_dtypes

from concourse import bass, mybir
import concourse.bacc as bacc
import concourse.tile as tile
from concourse.bass_utils import run_bass_kernel_spmd
from concourse.masks import make_identity

# model dims
B, T, E, H, DH = 2, 2048, 256, 8, 32
V = 50257
BT = B * T  # 4096
SCALE = 1.0 / float(np.float32(np.sqrt(DH)))
NCORES = 8
VC = 6400  # padded vocab shard per core (8*6400 = 51200 >= V)
VPAD = VC * NCORES
NEG = -1e30

F32 = mybir.dt.float32
BF16 = mybir.dt.bfloat16
I32 = mybir.dt.int32
AX = mybir.AxisListType.X
ALU = mybir.AluOpType
ACTF = mybir.ActivationFunctionType

_CACHE = {}


def _mktile(tc, *args, **kwargs):
    t, _free = tc.tile(*args, **kwargs)
    return t


def _build(has_bias: bool, repeat: int = 1):
    nc = bacc.Bacc("TRN2", debug=False, num_devices=NCORES)

    # ---- I/O ----
    idx_d = nc.dram_tensor("idxs", [128, 16], I32, kind="ExternalInput")
    temb_d = nc.dram_tensor("tok_emb", [V, E], F32, kind="ExternalInput")
    pos_d = nc.dram_tensor("pos", [T, E], F32, kind="ExternalInput")
    wq_d = nc.dram_tensor("wq", [2, E, DH], BF16, kind="ExternalInput")
    wk_d = nc.dram_tensor("wk", [2, E, DH], BF16, kind="ExternalInput")
    wv_d = nc.dram_tensor("wv", [2, E, DH], BF16, kind="ExternalInput")
    wlmT_d = nc.dram_tensor("wlmT", [E, VC], BF16, kind="ExternalInput")
    masks_d = nc.dram_tensor("masks", [4, 128, 512], F32, kind="ExternalInput")
    if has_bias:
        blm_d = nc.dram_tensor("blmsh", [1, VC], F32, kind="ExternalInput")
    logits_d = nc.dram_tensor("logits", [BT, VC], F32, kind="ExternalOutput")
    se_d = nc.dram_tensor("sumexp", [128, 32], F32, kind="ExternalOutput")

    with tile.TileContext(nc) as tc, \
            tc.tile_pool(name="glob", bufs=1) as gp, \
            tc.tile_pool(name="globd", bufs=1, space="DRAM") as gd:
        # ---------------- constants / weights resident in SBUF ----------------
        ident = gp.tile([128, 128], F32, name="ident", tag="ident")
        make_identity(nc, ident[:])

        masks_sb = gp.tile([128, 4 * 512], F32, name="masks_sb", tag="masks_sb")
        for r in range(4):
            nc.sync.dma_start(masks_sb[:, r * 512 : (r + 1) * 512], masks_d[r])

        idx_sb = gp.tile([128, 16], I32, name="idx_sb", tag="idx_sb")
        nc.sync.dma_start(idx_sb[:], idx_d[:])

        # per-head projection weights [128, (hh*2+kk)*DH] layout
        wq_sb = gp.tile([128, 4 * DH], BF16, name="wq_sb", tag="wq_sb")
        wk_sb = gp.tile([128, 4 * DH], BF16, name="wk_sb", tag="wk_sb")
        wv_sb = gp.tile([128, 4 * DH], BF16, name="wv_sb", tag="wv_sb")
        for hh in range(2):
            for kk in range(2):
                c0 = (hh * 2 + kk) * DH
                sl = slice(kk * 128, (kk + 1) * 128)
                nc.sync.dma_start(wq_sb[:, c0 : c0 + DH], wq_d[hh, sl, :])
                nc.sync.dma_start(wk_sb[:, c0 : c0 + DH], wk_d[hh, sl, :])
                nc.sync.dma_start(wv_sb[:, c0 : c0 + DH], wv_d[hh, sl, :])

        wlmT_sb = []
        for kk in range(2):
            t_ = gp.tile([128, VC], BF16, name=f"wlmT_sb{kk}", tag=f"wlmT_sb{kk}")
            nc.sync.dma_start(t_[:], wlmT_d[kk * 128 : (kk + 1) * 128, :])
            wlmT_sb.append(t_)

        if has_bias:
            ones_sb = gp.tile([1, 128], BF16, name="ones_sb", tag="ones_sb")
            nc.vector.memset(ones_sb[:], 1.0)
            blmf_sb = gp.tile([1, VC], F32, name="blmf_sb", tag="blmf_sb")
            nc.sync.dma_start(blmf_sb[:], blm_d[:])
            blm_sb = gp.tile([1, VC], BF16, name="blm_sb", tag="blm_sb")
            nc.vector.tensor_copy(blm_sb[:], blmf_sb[:])

        for _rep in range(repeat):
            _phases(nc, tc, gp, has_bias, locals())

    nc.compile()
    return nc


def _phases(nc, tc, gp, has_bias, env):
    gd = env["gd"]
    idx_sb = env["idx_sb"]; temb_d = env["temb_d"]; pos_d = env["pos_d"]
    ident = env["ident"]; masks_sb = env["masks_sb"]
    wq_sb = env["wq_sb"]; wk_sb = env["wk_sb"]; wv_sb = env["wv_sb"]
    wlmT_sb = env["wlmT_sb"]; logits_d = env["logits_d"]; se_d = env["se_d"]
    ones_sb = env.get("ones_sb"); blm_sb = env.get("blm_sb")
    if True:
        # ---------------- phase 1: embedding gather + transpose -> xT ----------------
        # xT_sb[kk] holds rows [kk*128,(kk+1)*128) of x^T  (E on partitions, this
        # core's 2048 tokens on free), bf16
        xT_sb = [gp.tile([128, T], BF16, name=f"xT{kk}", tag=f"xT{kk}") for kk in range(2)]

        with (
            tc.tile_pool(name="emb_sbuf", bufs=3) as ep,
            tc.tile_pool(name="emb_psum", bufs=2, space="PSUM") as epp,
        ):
            for n in range(16):
                xg = ep.tile([128, E], F32, tag="xg")
                nc.gpsimd.indirect_dma_start(
                    out=xg[:],
                    out_offset=None,
                    in_=temb_d[:],
                    in_offset=bass.IndirectOffsetOnAxis(ap=idx_sb[:, n : n + 1], axis=0),
                )
                pt = ep.tile([128, E], F32, tag="pos")
                nc.sync.dma_start(pt[:], pos_d[n * 128 : (n + 1) * 128, :])
                nc.vector.tensor_tensor(out=xg[:], in0=xg[:], in1=pt[:], op=ALU.add)
                for kk in range(2):
                    ps = epp.tile([128, 128], F32, tag="tp")
                    nc.tensor.transpose(
                        out=ps[:], in_=xg[:, kk * 128 : (kk + 1) * 128], identity=ident[:]
                    )
                    nc.vector.tensor_copy(
                        out=xT_sb[kk][:, n * 128 : (n + 1) * 128], in_=ps[:]
                    )

        # ---------------- phase 2: attention for this core's 2 heads ----------------
        att_bf = gp.tile([64, T], BF16, name="att_bf", tag="att_bf")  # rows: head hh -> [32hh,32hh+32)

        for hh in range(2):
            with tc.tile_pool(name=f"qkv_sb{hh}", bufs=1) as qs:
                qT = qs.tile([32, T], BF16, tag="qT")
                kT = qs.tile([32, T], BF16, tag="kT")
                v_bf = qs.tile([128, 16 * DH], BF16, tag="vbf")
                c0 = (hh * 2) * DH
                c1 = (hh * 2 + 1) * DH
                with tc.tile_pool(name=f"qkv_ps{hh}", bufs=2, space="PSUM") as qp:
                    for nn in range(4):
                        sl = slice(nn * 512, (nn + 1) * 512)
                        for dst, wsb in ((qT, wq_sb), (kT, wk_sb)):
                            ps = qp.tile([32, 512], F32, tag="pqk")
                            nc.tensor.matmul(
                                ps[:], wsb[:, c0 : c0 + DH], xT_sb[0][:, sl],
                                start=True, stop=False,
                            )
                            nc.tensor.matmul(
                                ps[:], wsb[:, c1 : c1 + DH], xT_sb[1][:, sl],
                                start=False, stop=True,
                            )
                            nc.vector.tensor_copy(out=dst[:, sl], in_=ps[:])
                    for m in range(16):
                        sl = slice(m * 128, (m + 1) * 128)
                        ps = qp.tile([128, DH], F32, tag="pv")
                        nc.tensor.matmul(
                            ps[:], xT_sb[0][:, sl], wv_sb[:, c0 : c0 + DH],
                            start=True, stop=False,
                        )
                        nc.tensor.matmul(
                            ps[:], xT_sb[1][:, sl], wv_sb[:, c1 : c1 + DH],
                            start=False, stop=True,
                        )
                        nc.vector.tensor_copy(
                            out=v_bf[:, m * DH : (m + 1) * DH], in_=ps[:]
                        )

                # scores -> exp (+ per-column sums) ; exp kept per k-block
                with (
                    tc.tile_pool(name=f"exp{hh}", bufs=16) as xp,
                    tc.tile_pool(name=f"sc_ps{hh}", bufs=3, space="PSUM") as sp,
                    tc.tile_pool(name=f"att_ps{hh}", bufs=2, space="PSUM") as op_,
                    tc.tile_pool(name=f"att_sb{hh}", bufs=1) as st,
                ):
                    cs_parts = st.tile([128, 64], F32, tag="csp")
                    nc.vector.memset(cs_parts[:], 0.0)
                    exp_tiles = []
                    for i in range(16):
                        et = xp.tile([128, T], BF16, tag="exp")
                        exp_tiles.append(et)
                        jmin = i // 4
                        for j in range(jmin, 4):
                            ps = sp.tile([128, 512], F32, tag="ps")
                            nc.tensor.matmul(
                                ps[:],
                                kT[:, i * 128 : (i + 1) * 128],
                                qT[:, j * 512 : (j + 1) * 512],
                                start=True, stop=True,
                            )
                            if j == jmin:
                                r = i % 4
                                nc.vector.tensor_tensor(
                                    out=ps[:], in0=ps[:],
                                    in1=masks_sb[:, r * 512 : (r + 1) * 512],
                                    op=ALU.add,
                                )
                            nc.scalar.activation(
                                out=et[:, j * 512 : (j + 1) * 512],
                                in_=ps[:],
                                func=ACTF.Exp,
                                scale=SCALE,
                                accum_out=cs_parts[:, 4 * i + j : 4 * i + j + 1],
                            )
                    colsum = st.tile([128, 16], F32, tag="cs")
                    nc.vector.tensor_reduce(
                        out=colsum[:],
                        in_=cs_parts[:].rearrange("p (i j) -> p i j", j=4),
                        axis=AX, op=ALU.add,
                    )
                    recip = st.tile([128, 16], F32, tag="rc")
                    nc.vector.reciprocal(out=recip[:], in_=colsum[:])
                    vs_bf = st.tile([128, 16 * DH], BF16, tag="vs")
                    for m in range(16):
                        nc.vector.tensor_scalar_mul(
                            out=vs_bf[:, m * DH : (m + 1) * DH],
                            in0=v_bf[:, m * DH : (m + 1) * DH],
                            scalar1=recip[:, m : m + 1],
                        )
                    for j in range(4):
                        po = op_.tile([32, 512], F32, tag="po")
                        last = 4 * j + 3
                        for i in range(last + 1):
                            nc.tensor.matmul(
                                po[:],
                                vs_bf[:, i * DH : (i + 1) * DH],
                                exp_tiles[i][:, j * 512 : (j + 1) * 512],
                                start=(i == 0), stop=(i == last),
                            )
                        nc.vector.tensor_copy(
                            out=att_bf[32 * hh : 32 * hh + 32, j * 512 : (j + 1) * 512],
                            in_=po[:],
                        )

        # ---------------- phase 3: all-gather head outputs ----------------
        att_dram = gd.tile([64, T], BF16, name="att_dram", tag="att_dram")
        nc.sync.dma_start(att_dram[:], att_bf[:])
        out_all = gd.tile([512, T], BF16, name="out_all", tag="out_all", addr_space="Shared")
        nc.gpsimd.collective_compute(
            "AllGather",
            ALU.bypass,
            replica_groups=[list(range(NCORES))],
            ins=[att_dram.opt()],
            outs=[out_all.opt()],
        )
        oall_sb = []
        for kb in range(4):
            t_ = gp.tile([128, T], BF16, name=f"oall{kb}", tag=f"oall{kb}")
            nc.sync.dma_start(t_[:], out_all[kb * 128 : (kb + 1) * 128, :])
            oall_sb.append(t_)

        # ---------------- phase 4: lm_head over vocab shard ----------------
        # v-col groups: 3 x 2048 + 1 x 256 (VC = 6400)
        GW = [2048, 2048, 2048, 256]
        GO = [0, 2048, 4096, 6144]
        se_sb = gp.tile([128, 32], F32, name="se_sb", tag="se_sb")
        with (
            tc.tile_pool(name="lm_ps", bufs=2, space="PSUM") as lp,
            tc.tile_pool(name="lm_stage", bufs=2) as lsp,
            tc.tile_pool(name="lm_scr", bufs=3) as scp,
        ):
            for n in range(32):
                b, nn = n // 16, n % 16
                lhs0 = oall_sb[2 * b][:, nn * 128 : (nn + 1) * 128]
                lhs1 = oall_sb[2 * b + 1][:, nn * 128 : (nn + 1) * 128]
                stage = lsp.tile([128, VC], F32, tag="stage")
                separt = scp.tile([128, 4], F32, tag="sep")
                for g in range(4):
                    w, off = GW[g], GO[g]
                    psl = lp.tile([128, 2048], F32, tag="psl")
                    for sub in range(0, w, 512):
                        sw = min(512, w - sub)
                        vo = off + sub
                        psl_sl = psl[:, sub : sub + sw]
                        nc.tensor.matmul(
                            psl_sl, lhs0, wlmT_sb[0][:, vo : vo + sw],
                            start=True, stop=False,
                        )
                        nc.tensor.matmul(
                            psl_sl, lhs1, wlmT_sb[1][:, vo : vo + sw],
                            start=False, stop=not has_bias,
                        )
                        if has_bias:
                            nc.tensor.matmul(
                                psl_sl, ones_sb[:], blm_sb[:, vo : vo + sw],
                                start=False, stop=True,
                            )
                    nc.vector.tensor_copy(
                        out=stage[:, off : off + w], in_=psl[:, :w]
                    )
                    escr = scp.tile([128, 2048], BF16, tag="escr")
                    nc.scalar.activation(
                        out=escr[:, :w],
                        in_=stage[:, off : off + w],
                        func=ACTF.Exp,
                        scale=1.0,
                        accum_out=separt[:, g : g + 1],
                    )
                nc.vector.tensor_reduce(
                    out=se_sb[:, n : n + 1], in_=separt[:], axis=AX, op=ALU.add
                )
                nc.sync.dma_start(logits_d[n * 128 : (n + 1) * 128, :], stage[:])
        nc.sync.dma_start(se_d[:], se_sb[:])


def _prep_inputs(idx, targets, tok_emb, pos_emb, Wq, Wk, Wv, Wlm, blm, has_bias):
    bf = ml_dtypes.bfloat16
    tok_emb = np.ascontiguousarray(tok_emb, np.float32)
    pos_emb = np.ascontiguousarray(pos_emb, np.float32)
    Wq = np.asarray(Wq, np.float32)
    Wk = np.asarray(Wk, np.float32)
    Wv = np.asarray(Wv, np.float32)
    Wlm_pad = np.zeros((VPAD, E), np.float32)
    Wlm_pad[:V] = np.asarray(Wlm, np.float32)
    blm_pad = np.full((VPAD,), NEG, np.float32)
    blm_pad[:V] = np.asarray(blm, np.float32)

    masks = np.empty((4, 128, 512), np.float32)
    yy, xx = np.meshgrid(np.arange(512), np.arange(128))  # xx: k within block, yy: q
    for r in range(4):
        masks[r] = np.where(yy - xx >= 128 * r, 0.0, NEG).astype(np.float32)

    in_maps = []
    for c in range(NCORES):
        b = c // 4
        h0 = 2 * (c % 4)
        idx_b = np.asarray(idx[b], np.int64).astype(np.int32)  # [2048]
        m = {
            "idxs": np.ascontiguousarray(idx_b.reshape(16, 128).T),
            "tok_emb": tok_emb,
            "pos": pos_emb,
            "wq": np.ascontiguousarray(Wq[h0 : h0 + 2].transpose(0, 1, 2)).astype(bf),
            "wk": np.ascontiguousarray(Wk[h0 : h0 + 2]).astype(bf),
            "wv": np.ascontiguousarray(Wv[h0 : h0 + 2]).astype(bf),
            "wlmT": np.ascontiguousarray(Wlm_pad[c * VC : (c + 1) * VC].T).astype(bf),
            "masks": masks,
        }
        if has_bias:
            m["blmsh"] = np.ascontiguousarray(blm_pad[c * VC : (c + 1) * VC][None, :])
        in_maps.append(m)
    return in_maps


def run(idx, targets, tok_emb, pos_emb, Wq, Wk, Wv, Wlm, blm, trace=False, repeat=1, **trace_kw):
    has_bias = bool(np.any(np.asarray(blm) != 0))
    key = (has_bias, repeat)
    if key not in _CACHE:
        _CACHE[key] = _build(has_bias, repeat)
    nc = _CACHE[key]
    in_maps = _prep_inputs(
        idx, targets, tok_emb, pos_emb, Wq, Wk, Wv, Wlm, blm, has_bias
    )
    res = run_bass_kernel_spmd(
        nc, in_maps, core_ids=list(range(NCORES)), trace=trace, **trace_kw
    )

    logits = np.concatenate(
        [np.asarray(res.results[c]["logits"]) for c in range(NCORES)], axis=1
    )[:, :V]
    se = np.zeros((BT,), np.float64)
    for c in range(NCORES):
        se += np.asarray(res.results[c]["sumexp"], np.float64).T.reshape(BT)
    if not has_bias:
        se -= float(VPAD - V)  # padded cols have logits exactly 0 -> exp == 1
    lse = np.log(se)
    tgt_flat = np.asarray(targets, np.int64).reshape(BT)
    tgt_logit = logits[np.arange(BT), tgt_flat].astype(np.float64)
    loss = np.float32(np.mean(lse - tgt_logit))
    return logits.reshape(B, T, V), loss, res


def kernel(idx, targets, tok_emb, pos_emb, Wq, Wk, Wv, Wlm, blm):
    logits, loss, _ = run(idx, targets, tok_emb, pos_emb, Wq, Wk, Wv, Wlm, blm)
    return logits, loss


# revision 14
# speedup vs baseline: 1.1309x; 1.1309x over previous
"""GPT forward (embed + 8-head causal attn w/ query-axis softmax + lm_head + CE loss)
on 8 Trainium2 NeuronCores.

Sharding: attention is sharded by (batch, head) pairs (2 per core); the head
outputs are all-gathered on device; the lm_head / logits / log-sum-exp are
sharded over the vocab axis (6400 padded rows per core). Host only
slices/concatenates shards and combines the per-shard sum-exp scalars.
"""

import sys

sys.path.insert(0, "/opt/trn_rl_repo")

import numpy as np
import ml_dtypes

from concourse import bass, mybir
import concourse.bacc as bacc
import concourse.tile as tile
from concourse.bass_utils import run_bass_kernel_spmd
from concourse.masks import make_identity

# model dims
B, T, E, H, DH = 2, 2048, 256, 8, 32
V = 50257
BT = B * T  # 4096
SCALE = 1.0 / float(np.float32(np.sqrt(DH)))
NCORES = 8
VC = 6400  # padded vocab shard per core (8*6400 = 51200 >= V)
VPAD = VC * NCORES
NEG = -1e30

F32 = mybir.dt.float32
BF16 = mybir.dt.bfloat16
I32 = mybir.dt.int32
AX = mybir.AxisListType.X
ALU = mybir.AluOpType
ACTF = mybir.ActivationFunctionType

_CACHE = {}


def _mktile(tc, *args, **kwargs):
    t, _free = tc.tile(*args, **kwargs)
    return t


def _build(has_bias: bool, repeat: int = 1, parts: str = "all"):
    nc = bacc.Bacc("TRN2", debug=False, num_devices=NCORES)

    # ---- I/O ----
    idx_d = nc.dram_tensor("idxs", [128, 16], I32, kind="ExternalInput")
    temb_d = nc.dram_tensor("tok_emb", [V, E], F32, kind="ExternalInput")
    posT_d = nc.dram_tensor("posT", [2, 128, T], F32, kind="ExternalInput")
    wq_d = nc.dram_tensor("wq", [2, E, DH], BF16, kind="ExternalInput")
    wk_d = nc.dram_tensor("wk", [2, E, DH], BF16, kind="ExternalInput")
    wv_d = nc.dram_tensor("wv", [2, E, DH], BF16, kind="ExternalInput")
    wlmT_d = nc.dram_tensor("wlmT", [E, VC], BF16, kind="ExternalInput")
    masks_d = nc.dram_tensor("masks", [4, 128, 512], F32, kind="ExternalInput")
    if has_bias:
        blm_d = nc.dram_tensor("blmsh", [1, VC], F32, kind="ExternalInput")
    logits_d = nc.dram_tensor("logits", [BT, VC], F32, kind="ExternalOutput")
    se_d = nc.dram_tensor("sumexp", [128, 32], F32, kind="ExternalOutput")

    with tile.TileContext(nc) as tc, \
            tc.tile_pool(name="glob", bufs=1) as gp, \
            tc.tile_pool(name="globd", bufs=1, space="DRAM") as gd:
        # ---------------- constants / weights resident in SBUF ----------------
        ident = gp.tile([128, 128], F32, name="ident", tag="ident")
        make_identity(nc, ident[:])

        masks_sb = gp.tile([128, 4 * 512], F32, name="masks_sb", tag="masks_sb")
        for r in range(4):
            nc.sync.dma_start(masks_sb[:, r * 512 : (r + 1) * 512], masks_d[r])

        idx_sb = gp.tile([128, 16], I32, name="idx_sb", tag="idx_sb")
        nc.sync.dma_start(idx_sb[:], idx_d[:])

        posT_sb = []
        for kk in range(2):
            pT = gp.tile([128, T], F32, name=f"posT_sb{kk}", tag=f"posT_sb{kk}")
            nc.sync.dma_start(pT[:], posT_d[kk])
            posT_sb.append(pT)

        # per-head projection weights [128, (hh*2+kk)*DH] layout
        wq_sb = gp.tile([128, 4 * DH], BF16, name="wq_sb", tag="wq_sb")
        wk_sb = gp.tile([128, 4 * DH], BF16, name="wk_sb", tag="wk_sb")
        wv_sb = gp.tile([128, 4 * DH], BF16, name="wv_sb", tag="wv_sb")
        for hh in range(2):
            for kk in range(2):
                c0 = (hh * 2 + kk) * DH
                sl = slice(kk * 128, (kk + 1) * 128)
                nc.sync.dma_start(wq_sb[:, c0 : c0 + DH], wq_d[hh, sl, :])
                nc.sync.dma_start(wk_sb[:, c0 : c0 + DH], wk_d[hh, sl, :])
                nc.sync.dma_start(wv_sb[:, c0 : c0 + DH], wv_d[hh, sl, :])

        wlmT_sb = []
        for kk in range(2):
            t_ = gp.tile([128, VC], BF16, name=f"wlmT_sb{kk}", tag=f"wlmT_sb{kk}")
            nc.sync.dma_start(t_[:], wlmT_d[kk * 128 : (kk + 1) * 128, :])
            wlmT_sb.append(t_)

        if has_bias:
            ones_sb = gp.tile([1, 128], BF16, name="ones_sb", tag="ones_sb")
            nc.vector.memset(ones_sb[:], 1.0)
            blmf_sb = gp.tile([1, VC], F32, name="blmf_sb", tag="blmf_sb")
            nc.sync.dma_start(blmf_sb[:], blm_d[:])
            blm_sb = gp.tile([1, VC], BF16, name="blm_sb", tag="blm_sb")
            nc.vector.tensor_copy(blm_sb[:], blmf_sb[:])

        env = locals()
        if parts == "all":
            for _rep in range(repeat):
                oall_sb = _attn_ag(nc, tc, gp, env)
                _lm(nc, tc, gp, has_bias, env, oall_sb)
        elif parts == "lm":
            oall_sb = _attn_ag(nc, tc, gp, env)
            for _rep in range(repeat):
                _lm(nc, tc, gp, has_bias, env, oall_sb)
        elif parts == "attn":
            for _rep in range(repeat):
                oall_sb = _attn_ag(nc, tc, gp, env)
            _lm(nc, tc, gp, has_bias, env, oall_sb)

    nc.compile()
    return nc


def _attn_ag(nc, tc, gp, env):
    gd = env["gd"]
    idx_sb = env["idx_sb"]; temb_d = env["temb_d"]
    ident = env["ident"]; masks_sb = env["masks_sb"]
    posT_sb = env["posT_sb"]
    wq_sb = env["wq_sb"]; wk_sb = env["wk_sb"]; wv_sb = env["wv_sb"]
    if True:
        # ---- phase 1: embedding gather + transpose -> xT (pos added during
        # PSUM evacuation, free: tensor_tensor costs the same as tensor_copy)
        xT_sb = [gp.tile([128, T], BF16, name=f"xT{kk}", tag=f"xT{kk}") for kk in range(2)]

        with (
            tc.tile_pool(name="emb_sbuf", bufs=3) as ep,
            tc.tile_pool(name="emb_psum", bufs=3, space="PSUM") as epp,
        ):
            for n in range(16):
                xg = ep.tile([128, E], F32, tag="xg")
                nc.gpsimd.indirect_dma_start(
                    out=xg[:],
                    out_offset=None,
                    in_=temb_d[:],
                    in_offset=bass.IndirectOffsetOnAxis(ap=idx_sb[:, n : n + 1], axis=0),
                )
                sl = slice(n * 128, (n + 1) * 128)
                for kk in range(2):
                    ps = epp.tile([128, 128], F32, tag="tp")
                    nc.tensor.transpose(
                        out=ps[:], in_=xg[:, kk * 128 : (kk + 1) * 128], identity=ident[:]
                    )
                    nc.vector.tensor_tensor(
                        out=xT_sb[kk][:, sl], in0=ps[:], in1=posT_sb[kk][:, sl],
                        op=ALU.add,
                    )

        # ---- phase 2: attention, fully pipelined per k-block ----
        att_bf = gp.tile([64, T], BF16, name="att_bf", tag="att_bf")

        for hh in range(2):
            with tc.tile_pool(name=f"qkv_sb{hh}", bufs=1) as qs:
                qT = qs.tile([32, T], BF16, tag="qT")
                kT = qs.tile([32, T], BF16, tag="kT")
                v_bf = qs.tile([128, 16 * DH], BF16, tag="vbf")
                c0 = (hh * 2) * DH
                c1 = (hh * 2 + 1) * DH
                with tc.tile_pool(name=f"qkv_ps{hh}", bufs=2, space="PSUM") as qp:
                    for nn in range(4):
                        sl = slice(nn * 512, (nn + 1) * 512)
                        for dst, wsb, ecopy in ((qT, wq_sb, 0), (kT, wk_sb, 1)):
                            ps = qp.tile([32, 512], F32, tag="pqk")
                            nc.tensor.matmul(
                                ps[:], wsb[:, c0 : c0 + DH], xT_sb[0][:, sl],
                                start=True, stop=False,
                            )
                            nc.tensor.matmul(
                                ps[:], wsb[:, c1 : c1 + DH], xT_sb[1][:, sl],
                                start=False, stop=True,
                            )
                            if ecopy:
                                nc.scalar.copy(dst[:, sl], ps[:])
                            else:
                                nc.vector.tensor_copy(out=dst[:, sl], in_=ps[:])
                    for m in range(16):
                        sl = slice(m * 128, (m + 1) * 128)
                        ps = qp.tile([128, DH], F32, tag="pv")
                        nc.tensor.matmul(
                            ps[:], xT_sb[0][:, sl], wv_sb[:, c0 : c0 + DH],
                            start=True, stop=False,
                        )
                        nc.tensor.matmul(
                            ps[:], xT_sb[1][:, sl], wv_sb[:, c1 : c1 + DH],
                            start=False, stop=True,
                        )
                        nc.vector.tensor_copy(
                            out=v_bf[:, m * DH : (m + 1) * DH], in_=ps[:]
                        )

                # per-k-block pipeline: scores -> (mask) -> exp+colsum ->
                # recip -> scale v -> attn@v accumulate. No global barrier.
                with (
                    tc.tile_pool(name=f"exp{hh}", bufs=4) as xp,
                    tc.tile_pool(name=f"sc_ps{hh}", bufs=3, space="PSUM") as sp,
                    tc.tile_pool(name=f"att_ps{hh}", bufs=1, space="PSUM") as op_,
                    tc.tile_pool(name=f"att_sb{hh}", bufs=1) as st,
                ):
                    cs_parts = st.tile([128, 64], F32, tag="csp")
                    nc.vector.memset(cs_parts[:], 0.0)
                    colsum = st.tile([128, 16], F32, tag="cs")
                    recip = st.tile([128, 16], F32, tag="rc")
                    vs_bf = st.tile([128, 16 * DH], BF16, tag="vs")
                    po = [op_.tile([32, 512], F32, tag=f"po{j}", name=f"po{j}") for j in range(4)]
                    for i in range(16):
                        jmin = i // 4
                        et = xp.tile([128, T], BF16, tag="exp")
                        for j in range(jmin, 4):
                            ps = sp.tile([128, 512], F32, tag="ps")
                            nc.tensor.matmul(
                                ps[:],
                                kT[:, i * 128 : (i + 1) * 128],
                                qT[:, j * 512 : (j + 1) * 512],
                                start=True, stop=True,
                            )
                            if j == jmin:
                                r = i % 4
                                nc.vector.tensor_tensor(
                                    out=ps[:], in0=ps[:],
                                    in1=masks_sb[:, r * 512 : (r + 1) * 512],
                                    op=ALU.add,
                                )
                            nc.scalar.activation(
                                out=et[:, j * 512 : (j + 1) * 512],
                                in_=ps[:],
                                func=ACTF.Exp,
                                scale=SCALE,
                                accum_out=cs_parts[:, 4 * i + j : 4 * i + j + 1],
                            )
                        nc.vector.tensor_reduce(
                            out=colsum[:, i : i + 1],
                            in_=cs_parts[:, 4 * i : 4 * i + 4],
                            axis=AX, op=ALU.add,
                        )
                        nc.vector.reciprocal(
                            out=recip[:, i : i + 1], in_=colsum[:, i : i + 1]
                        )
                        nc.vector.tensor_scalar_mul(
                            out=vs_bf[:, i * DH : (i + 1) * DH],
                            in0=v_bf[:, i * DH : (i + 1) * DH],
                            scalar1=recip[:, i : i + 1],
                        )
                        for j in range(jmin, 4):
                            nc.tensor.matmul(
                                po[j][:],
                                vs_bf[:, i * DH : (i + 1) * DH],
                                et[:, j * 512 : (j + 1) * 512],
                                start=(i == 0), stop=(i == 4 * j + 3),
                            )
                    for j in range(4):
                        nc.vector.tensor_copy(
                            out=att_bf[32 * hh : 32 * hh + 32, j * 512 : (j + 1) * 512],
                            in_=po[j][:],
                        )

        # ---- phase 3: all-gather ----
        att_dram = gd.tile([64, T], BF16, name="att_dram", tag="att_dram")
        nc.sync.dma_start(att_dram[:], att_bf[:])
        out_all = gd.tile([512, T], BF16, name="out_all", tag="out_all", addr_space="Shared")
        nc.gpsimd.collective_compute(
            "AllGather",
            ALU.bypass,
            replica_groups=[list(range(NCORES))],
            ins=[att_dram.opt()],
            outs=[out_all.opt()],
        )
        oall_sb = []
        for kb in range(4):
            t_ = gp.tile([128, T], BF16, name=f"oall{kb}", tag=f"oall{kb}")
            nc.sync.dma_start(t_[:], out_all[kb * 128 : (kb + 1) * 128, :])
            oall_sb.append(t_)

        return oall_sb


def _lm(nc, tc, gp, has_bias, env, oall_sb):
    wlmT_sb = env["wlmT_sb"]; logits_d = env["logits_d"]; se_d = env["se_d"]
    ones_sb = env.get("ones_sb"); blm_sb = env.get("blm_sb")
    if True:
        # ---------------- phase 4: lm_head over vocab shard ----------------
        # v-col groups: 3 x 2048 + 1 x 256 (VC = 6400)
        GW = [2048, 2048, 2048, 256]
        GO = [0, 2048, 4096, 6144]
        se_sb = gp.tile([128, 32], F32, name="se_sb", tag="se_sb")
        with (
            tc.tile_pool(name="lm_ps", bufs=2, space="PSUM") as lp,
            tc.tile_pool(name="lm_stage", bufs=2) as lsp,
            tc.tile_pool(name="lm_scr", bufs=3) as scp,
        ):
            for n in range(32):
                b, nn = n // 16, n % 16
                lhs0 = oall_sb[2 * b][:, nn * 128 : (nn + 1) * 128]
                lhs1 = oall_sb[2 * b + 1][:, nn * 128 : (nn + 1) * 128]
                stage = lsp.tile([128, VC], F32, tag="stage")
                separt = scp.tile([128, 4], F32, tag="sep")
                for g in range(4):
                    w, off = GW[g], GO[g]
                    psl = lp.tile([128, 2048], F32, tag="psl")
                    for sub in range(0, w, 512):
                        sw = min(512, w - sub)
                        vo = off + sub
                        psl_sl = psl[:, sub : sub + sw]
                        nc.tensor.matmul(
                            psl_sl, lhs0, wlmT_sb[0][:, vo : vo + sw],
                            start=True, stop=False,
                        )
                        nc.tensor.matmul(
                            psl_sl, lhs1, wlmT_sb[1][:, vo : vo + sw],
                            start=False, stop=not has_bias,
                        )
                        if has_bias:
                            nc.tensor.matmul(
                                psl_sl, ones_sb[:], blm_sb[:, vo : vo + sw],
                                start=False, stop=True,
                            )
                    nc.vector.tensor_copy(
                        out=stage[:, off : off + w], in_=psl[:, :w]
                    )
                    escr = scp.tile([128, 2048], BF16, tag="escr")
                    nc.scalar.activation(
                        out=escr[:, :w],
                        in_=stage[:, off : off + w],
                        func=ACTF.Exp,
                        scale=1.0,
                        accum_out=separt[:, g : g + 1],
                    )
                nc.vector.tensor_reduce(
                    out=se_sb[:, n : n + 1], in_=separt[:], axis=AX, op=ALU.add
                )
                nc.sync.dma_start(logits_d[n * 128 : (n + 1) * 128, :], stage[:])
        nc.sync.dma_start(se_d[:], se_sb[:])


def _prep_inputs(idx, targets, tok_emb, pos_emb, Wq, Wk, Wv, Wlm, blm, has_bias):
    bf = ml_dtypes.bfloat16
    tok_emb = np.ascontiguousarray(tok_emb, np.float32)
    posT = np.ascontiguousarray(
        np.asarray(pos_emb, np.float32).T.reshape(2, 128, T)
    )
    Wq = np.asarray(Wq, np.float32)
    Wk = np.asarray(Wk, np.float32)
    Wv = np.asarray(Wv, np.float32)
    Wlm_pad = np.zeros((VPAD, E), np.float32)
    Wlm_pad[:V] = np.asarray(Wlm, np.float32)
    blm_pad = np.full((VPAD,), NEG, np.float32)
    blm_pad[:V] = np.asarray(blm, np.float32)

    masks = np.empty((4, 128, 512), np.float32)
    yy, xx = np.meshgrid(np.arange(512), np.arange(128))  # xx: k within block, yy: q
    for r in range(4):
        masks[r] = np.where(yy - xx >= 128 * r, 0.0, NEG).astype(np.float32)

    in_maps = []
    for c in range(NCORES):
        b = c // 4
        h0 = 2 * (c % 4)
        idx_b = np.asarray(idx[b], np.int64).astype(np.int32)  # [2048]
        m = {
            "idxs": np.ascontiguousarray(idx_b.reshape(16, 128).T),
            "tok_emb": tok_emb,
            "posT": posT,
            "wq": np.ascontiguousarray(Wq[h0 : h0 + 2].transpose(0, 1, 2)).astype(bf),
            "wk": np.ascontiguousarray(Wk[h0 : h0 + 2]).astype(bf),
            "wv": np.ascontiguousarray(Wv[h0 : h0 + 2]).astype(bf),
            "wlmT": np.ascontiguousarray(Wlm_pad[c * VC : (c + 1) * VC].T).astype(bf),
            "masks": masks,
        }
        if has_bias:
            m["blmsh"] = np.ascontiguousarray(blm_pad[c * VC : (c + 1) * VC][None, :])
        in_maps.append(m)
    return in_maps


def run(idx, targets, tok_emb, pos_emb, Wq, Wk, Wv, Wlm, blm, trace=False, repeat=1, **trace_kw):
    has_bias = bool(np.any(np.asarray(blm) != 0))
    key = (has_bias, repeat)
    if key not in _CACHE:
        _CACHE[key] = _build(has_bias, repeat)
    nc = _CACHE[key]
    in_maps = _prep_inputs(
        idx, targets, tok_emb, pos_emb, Wq, Wk, Wv, Wlm, blm, has_bias
    )
    res = run_bass_kernel_spmd(
        nc, in_maps, core_ids=list(range(NCORES)), trace=trace, **trace_kw
    )

    logits = np.concatenate(
        [np.asarray(res.results[c]["logits"]) for c in range(NCORES)], axis=1
    )[:, :V]
    se = np.zeros((BT,), np.float64)
    for c in range(NCORES):
        se += np.asarray(res.results[c]["sumexp"], np.float64).T.reshape(BT)
    if not has_bias:
        se -= float(VPAD - V)  # padded cols have logits exactly 0 -> exp == 1
    lse = np.log(se)
    tgt_flat = np.asarray(targets, np.int64).reshape(BT)
    tgt_logit = logits[np.arange(BT), tgt_flat].astype(np.float64)
    loss = np.float32(np.mean(lse - tgt_logit))
    return logits.reshape(B, T, V), loss, res


def kernel(idx, targets, tok_emb, pos_emb, Wq, Wk, Wv, Wlm, blm):
    logits, loss, _ = run(idx, targets, tok_emb, pos_emb, Wq, Wk, Wv, Wlm, blm)
    return logits, loss


# revision 15
# speedup vs baseline: 1.5882x; 1.4043x over previous
"""GPT forward (embed + 8-head causal attn w/ query-axis softmax + lm_head + CE loss)
on 8 Trainium2 NeuronCores.

Sharding: attention is sharded by (batch, head) pairs (2 per core); the head
outputs are all-gathered on device; the lm_head / logits / log-sum-exp are
sharded over the vocab axis (6400 padded rows per core). Host only
slices/concatenates shards and combines the per-shard sum-exp scalars.
"""

import sys

sys.path.insert(0, "/opt/trn_rl_repo")

import numpy as np
import ml_dtypes

from concourse import bass, mybir
import concourse.bacc as bacc
import concourse.tile as tile
from concourse.bass_utils import run_bass_kernel_spmd
from concourse.masks import make_identity

# model dims
B, T, E, H, DH = 2, 2048, 256, 8, 32
V = 50257
BT = B * T  # 4096
SCALE = 1.0 / float(np.float32(np.sqrt(DH)))
NCORES = 8
VC = 6400  # padded vocab shard per core (8*6400 = 51200 >= V)
VPAD = VC * NCORES
NEG = -1e30

F32 = mybir.dt.float32
BF16 = mybir.dt.bfloat16
I32 = mybir.dt.int32
AX = mybir.AxisListType.X
ALU = mybir.AluOpType
ACTF = mybir.ActivationFunctionType

_CACHE = {}


def _mktile(tc, *args, **kwargs):
    t, _free = tc.tile(*args, **kwargs)
    return t


def _build(has_bias: bool, repeat: int = 1, parts: str = "all"):
    nc = bacc.Bacc("TRN2", debug=False, num_devices=NCORES)

    # ---- I/O ----
    idx_d = nc.dram_tensor("idxs", [128, 16], I32, kind="ExternalInput")
    temb_d = nc.dram_tensor("tok_emb", [V, E], F32, kind="ExternalInput")
    posT_d = nc.dram_tensor("posT", [2, 128, T], F32, kind="ExternalInput")
    wq_d = nc.dram_tensor("wq", [2, E, DH], BF16, kind="ExternalInput")
    wk_d = nc.dram_tensor("wk", [2, E, DH], BF16, kind="ExternalInput")
    wv_d = nc.dram_tensor("wv", [2, E, DH], BF16, kind="ExternalInput")
    wlmT_d = nc.dram_tensor("wlmT", [E, VC], BF16, kind="ExternalInput")
    masks_d = nc.dram_tensor("masks", [4, 128, 512], F32, kind="ExternalInput")
    if has_bias:
        blm_d = nc.dram_tensor("blmsh", [1, VC], F32, kind="ExternalInput")
    logits_d = nc.dram_tensor("logits", [BT, VC], BF16, kind="ExternalOutput")
    se_d = nc.dram_tensor("sumexp", [128, 32], F32, kind="ExternalOutput")

    with tile.TileContext(nc) as tc, \
            tc.tile_pool(name="glob", bufs=1) as gp, \
            tc.tile_pool(name="globd", bufs=1, space="DRAM") as gd:
        # ---------------- constants / weights resident in SBUF ----------------
        ident = gp.tile([128, 128], F32, name="ident", tag="ident")
        make_identity(nc, ident[:])

        masks_sb = gp.tile([128, 4 * 512], F32, name="masks_sb", tag="masks_sb")
        for r in range(4):
            nc.sync.dma_start(masks_sb[:, r * 512 : (r + 1) * 512], masks_d[r])

        idx_sb = gp.tile([128, 16], I32, name="idx_sb", tag="idx_sb")
        nc.sync.dma_start(idx_sb[:], idx_d[:])

        posT_sb = []
        for kk in range(2):
            pT = gp.tile([128, T], F32, name=f"posT_sb{kk}", tag=f"posT_sb{kk}")
            nc.sync.dma_start(pT[:], posT_d[kk])
            posT_sb.append(pT)

        # per-head projection weights [128, (hh*2+kk)*DH] layout
        wq_sb = gp.tile([128, 4 * DH], BF16, name="wq_sb", tag="wq_sb")
        wk_sb = gp.tile([128, 4 * DH], BF16, name="wk_sb", tag="wk_sb")
        wv_sb = gp.tile([128, 4 * DH], BF16, name="wv_sb", tag="wv_sb")
        for hh in range(2):
            for kk in range(2):
                c0 = (hh * 2 + kk) * DH
                sl = slice(kk * 128, (kk + 1) * 128)
                nc.sync.dma_start(wq_sb[:, c0 : c0 + DH], wq_d[hh, sl, :])
                nc.sync.dma_start(wk_sb[:, c0 : c0 + DH], wk_d[hh, sl, :])
                nc.sync.dma_start(wv_sb[:, c0 : c0 + DH], wv_d[hh, sl, :])

        wlmT_sb = []
        for kk in range(2):
            t_ = gp.tile([128, VC], BF16, name=f"wlmT_sb{kk}", tag=f"wlmT_sb{kk}")
            nc.sync.dma_start(t_[:], wlmT_d[kk * 128 : (kk + 1) * 128, :])
            wlmT_sb.append(t_)

        if has_bias:
            ones_sb = gp.tile([1, 128], BF16, name="ones_sb", tag="ones_sb")
            nc.vector.memset(ones_sb[:], 1.0)
            blmf_sb = gp.tile([1, VC], F32, name="blmf_sb", tag="blmf_sb")
            nc.sync.dma_start(blmf_sb[:], blm_d[:])
            blm_sb = gp.tile([1, VC], BF16, name="blm_sb", tag="blm_sb")
            nc.vector.tensor_copy(blm_sb[:], blmf_sb[:])

        env = locals()
        if parts == "all":
            for _rep in range(repeat):
                oall_sb = _attn_ag(nc, tc, gp, env)
                _lm(nc, tc, gp, has_bias, env, oall_sb)
        elif parts == "lm":
            oall_sb = _attn_ag(nc, tc, gp, env)
            for _rep in range(repeat):
                _lm(nc, tc, gp, has_bias, env, oall_sb)
        elif parts == "attn":
            for _rep in range(repeat):
                oall_sb = _attn_ag(nc, tc, gp, env)
            _lm(nc, tc, gp, has_bias, env, oall_sb)

    nc.compile()
    return nc


def _attn_ag(nc, tc, gp, env):
    gd = env["gd"]
    idx_sb = env["idx_sb"]; temb_d = env["temb_d"]
    ident = env["ident"]; masks_sb = env["masks_sb"]
    posT_sb = env["posT_sb"]
    wq_sb = env["wq_sb"]; wk_sb = env["wk_sb"]; wv_sb = env["wv_sb"]
    if True:
        # ---- phase 1: embedding gather + transpose -> xT (pos added during
        # PSUM evacuation, free: tensor_tensor costs the same as tensor_copy)
        xT_sb = [gp.tile([128, T], BF16, name=f"xT{kk}", tag=f"xT{kk}") for kk in range(2)]

        with (
            tc.tile_pool(name="emb_sbuf", bufs=3) as ep,
            tc.tile_pool(name="emb_psum", bufs=3, space="PSUM") as epp,
        ):
            for n in range(16):
                xg = ep.tile([128, E], F32, tag="xg")
                nc.gpsimd.indirect_dma_start(
                    out=xg[:],
                    out_offset=None,
                    in_=temb_d[:],
                    in_offset=bass.IndirectOffsetOnAxis(ap=idx_sb[:, n : n + 1], axis=0),
                )
                sl = slice(n * 128, (n + 1) * 128)
                for kk in range(2):
                    ps = epp.tile([128, 128], F32, tag="tp")
                    nc.tensor.transpose(
                        out=ps[:], in_=xg[:, kk * 128 : (kk + 1) * 128], identity=ident[:]
                    )
                    nc.vector.tensor_tensor(
                        out=xT_sb[kk][:, sl], in0=ps[:], in1=posT_sb[kk][:, sl],
                        op=ALU.add,
                    )

        # ---- phase 2: attention, fully pipelined per k-block ----
        att_bf = gp.tile([64, T], BF16, name="att_bf", tag="att_bf")

        for hh in range(2):
            with tc.tile_pool(name=f"qkv_sb{hh}", bufs=1) as qs:
                qT = qs.tile([32, T], BF16, tag="qT")
                kT = qs.tile([32, T], BF16, tag="kT")
                v_bf = qs.tile([128, 16 * DH], BF16, tag="vbf")
                c0 = (hh * 2) * DH
                c1 = (hh * 2 + 1) * DH
                with tc.tile_pool(name=f"qkv_ps{hh}", bufs=2, space="PSUM") as qp:
                    for nn in range(4):
                        sl = slice(nn * 512, (nn + 1) * 512)
                        for dst, wsb, ecopy in ((qT, wq_sb, 0), (kT, wk_sb, 1)):
                            ps = qp.tile([32, 512], F32, tag="pqk")
                            nc.tensor.matmul(
                                ps[:], wsb[:, c0 : c0 + DH], xT_sb[0][:, sl],
                                start=True, stop=False,
                            )
                            nc.tensor.matmul(
                                ps[:], wsb[:, c1 : c1 + DH], xT_sb[1][:, sl],
                                start=False, stop=True,
                            )
                            if ecopy:
                                nc.scalar.copy(dst[:, sl], ps[:])
                            else:
                                nc.vector.tensor_copy(out=dst[:, sl], in_=ps[:])
                    for m in range(16):
                        sl = slice(m * 128, (m + 1) * 128)
                        ps = qp.tile([128, DH], F32, tag="pv")
                        nc.tensor.matmul(
                            ps[:], xT_sb[0][:, sl], wv_sb[:, c0 : c0 + DH],
                            start=True, stop=False,
                        )
                        nc.tensor.matmul(
                            ps[:], xT_sb[1][:, sl], wv_sb[:, c1 : c1 + DH],
                            start=False, stop=True,
                        )
                        nc.vector.tensor_copy(
                            out=v_bf[:, m * DH : (m + 1) * DH], in_=ps[:]
                        )

                # per-k-block pipeline: scores -> (mask) -> exp+colsum ->
                # recip -> scale v -> attn@v accumulate. No global barrier.
                with (
                    tc.tile_pool(name=f"exp{hh}", bufs=4) as xp,
                    tc.tile_pool(name=f"sc_ps{hh}", bufs=3, space="PSUM") as sp,
                    tc.tile_pool(name=f"att_ps{hh}", bufs=1, space="PSUM") as op_,
                    tc.tile_pool(name=f"att_sb{hh}", bufs=1) as st,
                ):
                    cs_parts = st.tile([128, 64], F32, tag="csp")
                    nc.vector.memset(cs_parts[:], 0.0)
                    colsum = st.tile([128, 16], F32, tag="cs")
                    recip = st.tile([128, 16], F32, tag="rc")
                    vs_bf = st.tile([128, 16 * DH], BF16, tag="vs")
                    po = [op_.tile([32, 512], F32, tag=f"po{j}", name=f"po{j}") for j in range(4)]
                    for i in range(16):
                        jmin = i // 4
                        et = xp.tile([128, T], BF16, tag="exp")
                        for j in range(jmin, 4):
                            ps = sp.tile([128, 512], F32, tag="ps")
                            nc.tensor.matmul(
                                ps[:],
                                kT[:, i * 128 : (i + 1) * 128],
                                qT[:, j * 512 : (j + 1) * 512],
                                start=True, stop=True,
                            )
                            if j == jmin:
                                r = i % 4
                                nc.vector.tensor_tensor(
                                    out=ps[:], in0=ps[:],
                                    in1=masks_sb[:, r * 512 : (r + 1) * 512],
                                    op=ALU.add,
                                )
                            nc.scalar.activation(
                                out=et[:, j * 512 : (j + 1) * 512],
                                in_=ps[:],
                                func=ACTF.Exp,
                                scale=SCALE,
                                accum_out=cs_parts[:, 4 * i + j : 4 * i + j + 1],
                            )
                        nc.vector.tensor_reduce(
                            out=colsum[:, i : i + 1],
                            in_=cs_parts[:, 4 * i : 4 * i + 4],
                            axis=AX, op=ALU.add,
                        )
                        nc.vector.reciprocal(
                            out=recip[:, i : i + 1], in_=colsum[:, i : i + 1]
                        )
                        nc.vector.tensor_scalar_mul(
                            out=vs_bf[:, i * DH : (i + 1) * DH],
                            in0=v_bf[:, i * DH : (i + 1) * DH],
                            scalar1=recip[:, i : i + 1],
                        )
                        for j in range(jmin, 4):
                            nc.tensor.matmul(
                                po[j][:],
                                vs_bf[:, i * DH : (i + 1) * DH],
                                et[:, j * 512 : (j + 1) * 512],
                                start=(i == 0), stop=(i == 4 * j + 3),
                            )
                    for j in range(4):
                        nc.vector.tensor_copy(
                            out=att_bf[32 * hh : 32 * hh + 32, j * 512 : (j + 1) * 512],
                            in_=po[j][:],
                        )

        # ---- phase 3: all-gather ----
        att_dram = gd.tile([64, T], BF16, name="att_dram", tag="att_dram")
        nc.sync.dma_start(att_dram[:], att_bf[:])
        out_all = gd.tile([512, T], BF16, name="out_all", tag="out_all", addr_space="Shared")
        nc.gpsimd.collective_compute(
            "AllGather",
            ALU.bypass,
            replica_groups=[list(range(NCORES))],
            ins=[att_dram.opt()],
            outs=[out_all.opt()],
        )
        oall_sb = []
        for kb in range(4):
            t_ = gp.tile([128, T], BF16, name=f"oall{kb}", tag=f"oall{kb}")
            nc.sync.dma_start(t_[:], out_all[kb * 128 : (kb + 1) * 128, :])
            oall_sb.append(t_)

        return oall_sb


def _lm(nc, tc, gp, has_bias, env, oall_sb):
    wlmT_sb = env["wlmT_sb"]; logits_d = env["logits_d"]; se_d = env["se_d"]
    ones_sb = env.get("ones_sb"); blm_sb = env.get("blm_sb")
    if True:
        # ---------------- phase 4: lm_head over vocab shard ----------------
        # v-col groups: 3 x 2048 + 1 x 256 (VC = 6400)
        GW = [2048, 2048, 2048, 256]
        GO = [0, 2048, 4096, 6144]
        se_sb = gp.tile([128, 32], F32, name="se_sb", tag="se_sb")
        with (
            tc.tile_pool(name="lm_ps", bufs=2, space="PSUM") as lp,
            tc.tile_pool(name="lm_stage", bufs=2) as lsp,
            tc.tile_pool(name="lm_scr", bufs=3) as scp,
        ):
            for n in range(32):
                b, nn = n // 16, n % 16
                lhs0 = oall_sb[2 * b][:, nn * 128 : (nn + 1) * 128]
                lhs1 = oall_sb[2 * b + 1][:, nn * 128 : (nn + 1) * 128]
                stage = lsp.tile([128, VC], BF16, tag="stage")
                separt = scp.tile([128, 4], F32, tag="sep")
                for g in range(4):
                    w, off = GW[g], GO[g]
                    psl = lp.tile([128, 2048], F32, tag="psl")
                    for sub in range(0, w, 512):
                        sw = min(512, w - sub)
                        vo = off + sub
                        psl_sl = psl[:, sub : sub + sw]
                        nc.tensor.matmul(
                            psl_sl, lhs0, wlmT_sb[0][:, vo : vo + sw],
                            start=True, stop=False,
                        )
                        nc.tensor.matmul(
                            psl_sl, lhs1, wlmT_sb[1][:, vo : vo + sw],
                            start=False, stop=not has_bias,
                        )
                        if has_bias:
                            nc.tensor.matmul(
                                psl_sl, ones_sb[:], blm_sb[:, vo : vo + sw],
                                start=False, stop=True,
                            )
                    nc.vector.tensor_copy(
                        out=stage[:, off : off + w], in_=psl[:, :w]
                    )
                    escr = scp.tile([128, 2048], BF16, tag="escr")
                    nc.scalar.activation(
                        out=escr[:, :w],
                        in_=stage[:, off : off + w],
                        func=ACTF.Exp,
                        scale=1.0,
                        accum_out=separt[:, g : g + 1],
                    )
                nc.vector.tensor_reduce(
                    out=se_sb[:, n : n + 1], in_=separt[:], axis=AX, op=ALU.add
                )
                nc.sync.dma_start(logits_d[n * 128 : (n + 1) * 128, :], stage[:])
        nc.sync.dma_start(se_d[:], se_sb[:])


def _prep_inputs(idx, targets, tok_emb, pos_emb, Wq, Wk, Wv, Wlm, blm, has_bias):
    bf = ml_dtypes.bfloat16
    tok_emb = np.ascontiguousarray(tok_emb, np.float32)
    posT = np.ascontiguousarray(
        np.asarray(pos_emb, np.float32).T.reshape(2, 128, T)
    )
    Wq = np.asarray(Wq, np.float32)
    Wk = np.asarray(Wk, np.float32)
    Wv = np.asarray(Wv, np.float32)
    Wlm_pad = np.zeros((VPAD, E), np.float32)
    Wlm_pad[:V] = np.asarray(Wlm, np.float32)
    blm_pad = np.full((VPAD,), NEG, np.float32)
    blm_pad[:V] = np.asarray(blm, np.float32)

    masks = np.empty((4, 128, 512), np.float32)
    yy, xx = np.meshgrid(np.arange(512), np.arange(128))  # xx: k within block, yy: q
    for r in range(4):
        masks[r] = np.where(yy - xx >= 128 * r, 0.0, NEG).astype(np.float32)

    in_maps = []
    for c in range(NCORES):
        b = c // 4
        h0 = 2 * (c % 4)
        idx_b = np.asarray(idx[b], np.int64).astype(np.int32)  # [2048]
        m = {
            "idxs": np.ascontiguousarray(idx_b.reshape(16, 128).T),
            "tok_emb": tok_emb,
            "posT": posT,
            "wq": np.ascontiguousarray(Wq[h0 : h0 + 2].transpose(0, 1, 2)).astype(bf),
            "wk": np.ascontiguousarray(Wk[h0 : h0 + 2]).astype(bf),
            "wv": np.ascontiguousarray(Wv[h0 : h0 + 2]).astype(bf),
            "wlmT": np.ascontiguousarray(Wlm_pad[c * VC : (c + 1) * VC].T).astype(bf),
            "masks": masks,
        }
        if has_bias:
            m["blmsh"] = np.ascontiguousarray(blm_pad[c * VC : (c + 1) * VC][None, :])
        in_maps.append(m)
    return in_maps


def run(idx, targets, tok_emb, pos_emb, Wq, Wk, Wv, Wlm, blm, trace=False, repeat=1, **trace_kw):
    has_bias = bool(np.any(np.asarray(blm) != 0))
    key = (has_bias, repeat)
    if key not in _CACHE:
        _CACHE[key] = _build(has_bias, repeat)
    nc = _CACHE[key]
    in_maps = _prep_inputs(
        idx, targets, tok_emb, pos_emb, Wq, Wk, Wv, Wlm, blm, has_bias
    )
    res = run_bass_kernel_spmd(
        nc, in_maps, core_ids=list(range(NCORES)), trace=trace, **trace_kw
    )

    logits = np.concatenate(
        [np.asarray(res.results[c]["logits"]).astype(np.float32) for c in range(NCORES)],
        axis=1,
    )[:, :V]
    se = np.zeros((BT,), np.float64)
    for c in range(NCORES):
        se += np.asarray(res.results[c]["sumexp"], np.float64).T.reshape(BT)
    if not has_bias:
        se -= float(VPAD - V)  # padded cols have logits exactly 0 -> exp == 1
    lse = np.log(se)
    tgt_flat = np.asarray(targets, np.int64).reshape(BT)
    tgt_logit = logits[np.arange(BT), tgt_flat].astype(np.float64)
    loss = np.float32(np.mean(lse - tgt_logit))
    return logits.reshape(B, T, V), loss, res


def kernel(idx, targets, tok_emb, pos_emb, Wq, Wk, Wv, Wlm, blm):
    logits, loss, _ = run(idx, targets, tok_emb, pos_emb, Wq, Wk, Wv, Wlm, blm)
    return logits, loss
